# revision 15
# baseline (speedup 1.0000x reference)
"""
LongConvolution (causal FFT conv) Trainium2 Bass kernel — v3.

Problem: x (4, 8192, 1024) f32, filt (1024, 8192) f32.
  y[b, l, c] = sum_m x[b, m, c] * filt[c, l - m]   (causal, per-channel)
Reference computes this via zero-padded FFT of size N = 16384 = 128*128.

Strategy
--------
1. Packed-complex batches: z = x[2p] + i*x[2p+1].  The filter multiply is
   linear, so IFFT(FFT(z) * K) = y[2p] + i*y[2p+1] with NO Hermitian
   unpacking.  4 real convs become 2 complex pipelines: halves F2/I1
   matmul work and ALL twiddle/pointwise elementwise work.
2. f16 everywhere on-chip: matmuls at 1 cycle/row, DVE elementwise at
   2x rate.  PSUM stays f32.
3. All 18 elementwise ops on DVE.  (v2 offloaded 4 to GpSimd: SBUF port
   contention stretched concurrent DVE ops to GpSimd's duration — net
   loss.  ACT does only PSUM->SBUF converting copies.)
4. Stacked stationaries kill matmuls+LDWEIGHTS: F1 contracts the
   stacked [z_re; z_im] (K=128, one matmul per packed signal); I2 uses
   [gc|gs] / [-gs|gc] so Re and Im outputs come from one matmul pair,
   with two channels sharing a PSUM bank along the free dim.
5. 4-channel iterations: elementwise free-size 1024; per-channel PSUM
   tiles (1 bank) so all stages double-buffer in 8 banks.

Four-step FFT (k = 128*k2 + k1, n = 128*n1 + n2):
  A[n2,k1] = sum_n1 e^{-2pi i n1 k1/128} z[128 n1 + n2]     (F1, K=64x2)
  B = A * T1,  T1[n2,k1] = e^{-2pi i n2 k1/N}               (T1, DVE)
  R[k2,k1] = sum_n2 e^{-2pi i n2 k2/128} B[n2,k1]           (F2)
  P = R * K   (filter spectrum, host-precomputed, [k2,k1])  (PW)
  C[k1,n2] = sum_k2 e^{+2pi i n2 k2/128} P[k2,k1]           (I1)
  C' = C * conj(T1)  ([k1,n2] layout, T1 symmetric)         (T2)
  y[n1,n2] = sum_k1 e^{+2pi i n1 k1/128} C'[k1,n2], n1<64   (I2)
  y[2p] = Re, y[2p+1] = Im.

Sharding: d_model across the 8 cores (128 channels each).
"""

import os
import sys

import numpy as np

for p in ("/opt/trn_rl_repo",):
    if p not in sys.path:
        sys.path.insert(0, p)

os.environ.setdefault("MYCRO_LOCAL_CACHE", "1")

# ----------------------------------------------------------------------------
# configuration
# ----------------------------------------------------------------------------
B, L, D = 4, 8192, 1024
NFFT = 2 * L               # 16384 = 128 * 128
NC = 8                     # cores
CPC = D // NC              # channels per core = 128
CHG = 8                    # channels per iteration
NIT = CPC // CHG           # 32 iterations


def _consts():
    j = np.arange(128)
    ang128 = 2 * np.pi * np.outer(j, j) / 128
    angN = 2 * np.pi * np.outer(j, j) / NFFT
    return {
        "F_cos": np.cos(ang128), "F_sin": np.sin(ang128),
        "Tw_cos": np.cos(angN), "Tw_sin": np.sin(angN),
    }


def _build_program():
    import concourse.bacc as bacc
    import concourse.mybir as mybir
    from concourse import tile

    f32 = mybir.dt.float32
    f16 = mybir.dt.float16

    nc = bacc.Bacc(None, target_bir_lowering=False, debug=False)

    # --- DRAM I/O (all f16) ---
    # xw[it, (ri,n1), 2*chl+pk, n2] — stacked z_re/z_im chunks
    xw = nc.dram_tensor("xw", (NIT, 128, 2 * CHG, 128), f16, kind="ExternalInput")
    # kk[it, k2, chl, ri, k1]
    kk = nc.dram_tensor("kk", (NIT, 128, CHG, 2, 128), f16, kind="ExternalInput")
    f1m_d = nc.dram_tensor("f1m", (128, 256), f16, kind="ExternalInput")
    f2c_d = nc.dram_tensor("f2c", (128, 128), f16, kind="ExternalInput")
    f2s_d = nc.dram_tensor("f2s", (128, 128), f16, kind="ExternalInput")
    f2sn_d = nc.dram_tensor("f2sn", (128, 128), f16, kind="ExternalInput")
    fim_d = nc.dram_tensor("fim", (128, 512), f16, kind="ExternalInput")
    gcs_d = nc.dram_tensor("gcs", (128, 128), f16, kind="ExternalInput")
    gnc_d = nc.dram_tensor("gnc", (128, 128), f16, kind="ExternalInput")
    t1re_d = nc.dram_tensor("t1re", (128, 2, 128), f16, kind="ExternalInput")
    t1im_d = nc.dram_tensor("t1im", (128, 2, 128), f16, kind="ExternalInput")
    gncn_d = nc.dram_tensor("gncn", (128, 128), f16, kind="ExternalInput")
    # yw[pair, cl, (sig,n1), pk, n2] — two channels share one PSUM bank
    yw = nc.dram_tensor(
        "yw", (CPC // 2, 2, 128, 2, 128), f16, kind="ExternalOutput"
    )

    with tile.TileContext(nc) as tc:
        with (
            tc.tile_pool(name="const", bufs=1) as constp,
            tc.tile_pool(name="m", bufs=2) as mp,
            tc.tile_pool(name="kf", bufs=2) as kp,
            tc.tile_pool(name="work", bufs=2) as wp,
            tc.tile_pool(name="scr", bufs=2) as up,
            tc.tile_pool(name="out", bufs=3) as op,
            tc.tile_pool(name="pa", bufs=2, space="PSUM") as pap,
            tc.tile_pool(name="pr", bufs=2, space="PSUM") as prp,
            tc.tile_pool(name="pc", bufs=2, space="PSUM") as pcp,
            tc.tile_pool(name="py", bufs=2, space="PSUM") as pyp,
        ):
            f1m = constp.tile([128, 256], f16)
            f2c = constp.tile([128, 128], f16)
            f2s = constp.tile([128, 128], f16)
            f2sn = constp.tile([128, 128], f16)
            fim = constp.tile([128, 512], f16)
            gcs = constp.tile([128, 128], f16)
            gnc = constp.tile([128, 128], f16)
            gncn = constp.tile([128, 128], f16)
            t1re = constp.tile([128, 2, 128], f16)
            t1im = constp.tile([128, 2, 128], f16)
            nc.sync.dma_start(f1m[:], f1m_d[:])
            for t, d in (
                (t1re, t1re_d), (t1im, t1im_d), (f2c, f2c_d),
                (f2s, f2s_d), (f2sn, f2sn_d), (fim, fim_d),
                (gcs, gcs_d), (gnc, gnc_d), (gncn, gncn_d),
            ):
                nc.scalar.dma_start(t[:], d[:])
            t1re_b = (
                t1re[:].rearrange("p r (s n) -> p r s n", s=1)
                .broadcast_to([128, 2, 2 * CHG, 128])
            )
            t1im_b = (
                t1im[:].rearrange("p r (s n) -> p r s n", s=1)
                .broadcast_to([128, 2, 2 * CHG, 128])
            )
            t1re_bh = (
                t1re[:].rearrange("p r (s n) -> p r s n", s=1)
                .broadcast_to([128, 2, CHG, 128])
            )
            t1im_bh = (
                t1im[:].rearrange("p r (s n) -> p r s n", s=1)
                .broadcast_to([128, 2, CHG, 128])
            )

            # Per-iteration stage emitters.  Stages of iteration pairs are
            # emitted interleaved (2-wide software pipelining) so every
            # engine's in-order stream has independent work between
            # dependent stages of one iteration.
            st = {}  # it -> dict of live tiles

            def e_dma(it):
                s = st[it] = {}
                s["m4"] = mp.tile([128, 2 * CHG, 128], f16, tag="m", name="m4")
                nc.sync.dma_start(s["m4"][:], xw[it])
                s["kt"] = kp.tile([128, CHG, 2, 128], f16, tag="k", name="kt")
                nc.sync.dma_start(s["kt"][:], kk[it])

            def e_f1(it):
                s = st[it]
                s["asrc"] = wp.tile([128, 2, 2 * CHG, 128], f16, tag="asrc", name="asrc")
                for ch in range(CHG):
                    pa = pap.tile([128, 2, 2, 128], f32, tag="pa")
                    for pk in range(2):
                        nc.tensor.matmul(
                            pa[:, :, pk, :], s["m4"][:, 2 * ch + pk, :],
                            f1m[:], start=True, stop=True,
                        )
                    nc.scalar.copy(
                        out=s["asrc"][:, :, 2 * ch : 2 * ch + 2, :], in_=pa[:]
                    )

            def e_t1(it):
                s = st[it]
                b_t = s["b"] = wp.tile([128, 2, 2 * CHG, 128], f16, tag="b", name="b_t")
                u14 = up.tile([128, 2, 2 * CHG, 128], f16, tag="u14", name="u14")
                u23 = up.tile([128, 2, 2 * CHG, 128], f16, tag="u23", name="u23")
                # u14 = [a_re; a_im] * t1re ; u23 = [a_re; a_im] * t1im
                nc.vector.tensor_mul(u14[:], s["asrc"][:], t1re_b)
                nc.vector.tensor_mul(u23[:], s["asrc"][:], t1im_b)
                nc.vector.tensor_sub(
                    b_t[:, 0, :, :], u14[:, 0, :, :], u23[:, 1, :, :]
                )
                nc.vector.tensor_add(
                    b_t[:, 1, :, :], u23[:, 0, :, :], u14[:, 1, :, :]
                )

            def e_f2(it):
                s = st[it]
                b_t = s["b"]
                s["rsrc"] = wp.tile([128, 2, 2 * CHG, 128], f16, tag="rsrc", name="rsrc")
                for ch in range(CHG):
                    pr = prp.tile([128, 2, 2, 128], f32, tag="pr")
                    for pk in range(2):
                        j = 2 * ch + pk
                        b_re = b_t[:, 0, j, :]
                        b_im = b_t[:, 1, j, :]
                        nc.tensor.matmul(
                            pr[:, 0, pk, :], f2c[:], b_re,
                            start=True, stop=False,
                        )
                        nc.tensor.matmul(
                            pr[:, 0, pk, :], f2s[:], b_im,
                            start=False, stop=True,
                        )
                        nc.tensor.matmul(
                            pr[:, 1, pk, :], f2c[:], b_im,
                            start=True, stop=False,
                        )
                        nc.tensor.matmul(
                            pr[:, 1, pk, :], f2sn[:], b_re,
                            start=False, stop=True,
                        )
                    nc.scalar.copy(
                        out=s["rsrc"][:, :, 2 * ch : 2 * ch + 2, :], in_=pr[:]
                    )

            def e_pw(it):
                s = st[it]
                rsrc, kt = s["rsrc"], s["kt"]
                r_re = rsrc[:, 0, :, :].rearrange("p (c k) n -> p c k n", c=CHG)
                r_im = rsrc[:, 1, :, :].rearrange("p (c k) n -> p c k n", c=CHG)
                kre_b = (
                    kt[:, :, 0, :].rearrange("p c (s n) -> p c s n", s=1)
                    .broadcast_to([128, CHG, 2, 128])
                )
                kim_b = (
                    kt[:, :, 1, :].rearrange("p c (s n) -> p c s n", s=1)
                    .broadcast_to([128, CHG, 2, 128])
                )
                p_t = s["p"] = wp.tile([128, 2, 2 * CHG, 128], f16, tag="p", name="p_t")
                p_re = p_t[:, 0, :, :].rearrange("p (c k) n -> p c k n", c=CHG)
                p_im = p_t[:, 1, :, :].rearrange("p (c k) n -> p c k n", c=CHG)
                v1 = up.tile([128, CHG, 2, 128], f16, tag="u1")
                v2 = up.tile([128, CHG, 2, 128], f16, tag="u2")
                v3 = up.tile([128, CHG, 2, 128], f16, tag="u3")
                v4 = up.tile([128, CHG, 2, 128], f16, tag="u4")
                nc.vector.tensor_mul(v1[:], r_re, kre_b)
                nc.vector.tensor_mul(v2[:], r_im, kim_b)
                nc.vector.tensor_sub(p_re, v1[:], v2[:])
                nc.vector.tensor_mul(v3[:], r_re, kim_b)
                nc.vector.tensor_mul(v4[:], r_im, kre_b)
                nc.vector.tensor_add(p_im, v3[:], v4[:])

            def e_i1(it):
                s = st[it]
                p_t = s["p"]
                s["csrc"] = wp.tile([128, 2, 2 * CHG, 128], f16, tag="csrc", name="csrc")
                for ch in range(CHG):
                    pc = pcp.tile([128, 2, 2, 128], f32, tag="pc")
                    for pk in range(2):
                        j = 2 * ch + pk
                        nc.tensor.matmul(
                            pc[:, :, pk, :], p_t[:, 0, j, :], fim[:, 0:256],
                            start=True, stop=False,
                        )
                        nc.tensor.matmul(
                            pc[:, :, pk, :], p_t[:, 1, j, :], fim[:, 256:512],
                            start=False, stop=True,
                        )
                    nc.scalar.copy(
                        out=s["csrc"][:, :, 2 * ch : 2 * ch + 2, :], in_=pc[:]
                    )

            def e_t2_half(it, h):
                s = st[it]
                sl = slice(CHG * h, CHG * (h + 1))
                if h == 0:
                    s["w14"] = wp.tile(
                        [128, 2, 2 * CHG, 128], f16, tag="w14", name="w14"
                    )
                    s["w23"] = wp.tile(
                        [128, 2, 2 * CHG, 128], f16, tag="w23", name="w23"
                    )
                nc.vector.tensor_mul(
                    s["w14"][:, :, sl, :], s["csrc"][:, :, sl, :], t1re_bh
                )
                nc.vector.tensor_mul(
                    s["w23"][:, :, sl, :], s["csrc"][:, :, sl, :], t1im_bh
                )

            def e_t2(it):
                # w14 = [c_re; c_im] * t1re -> (w1, w4)
                # w23 = [c_re; c_im] * t1im -> (w3, w2)
                # cp_re = w1 + w2, cp_im = w4 - w3: folded into I2 matmuls
                s = st[it]
                w14 = s["w14"] = wp.tile(
                    [128, 2, 2 * CHG, 128], f16, tag="w14", name="w14"
                )
                w23 = s["w23"] = wp.tile(
                    [128, 2, 2 * CHG, 128], f16, tag="w23", name="w23"
                )
                nc.vector.tensor_mul(w14[:], s["csrc"][:], t1re_b)
                nc.vector.tensor_mul(w23[:], s["csrc"][:], t1im_b)

            def e_i2(it, chs=None):
                s = st[it]
                w14, w23 = s["w14"], s["w23"]
                py = None
                for ch in (chs if chs is not None else range(CHG)):
                    cl = ch % 2
                    if cl == 0:
                        py = pyp.tile([128, 2, 2, 128], f32, tag="py")
                    for pk in range(2):
                        j = 2 * ch + pk
                        # y = gcs@(w1+w2) + gnc@(w4-w3)
                        nc.tensor.matmul(
                            py[:, cl, pk, :], gcs[:], w14[:, 0, j, :],
                            start=True, stop=False,
                        )
                        nc.tensor.matmul(
                            py[:, cl, pk, :], gcs[:], w23[:, 1, j, :],
                            start=False, stop=False,
                        )
                        nc.tensor.matmul(
                            py[:, cl, pk, :], gnc[:], w14[:, 1, j, :],
                            start=False, stop=False,
                        )
                        nc.tensor.matmul(
                            py[:, cl, pk, :], gncn[:], w23[:, 0, j, :],
                            start=False, stop=True,
                        )
                    if cl == 1:
                        pair = (CHG * it + ch) // 2
                        ysb = op.tile([128, 2, 2, 128], f16, tag="ysb")
                        nc.scalar.copy(out=ysb[:], in_=py[:])
                        nc.sync.dma_start(
                            yw[pair].rearrange("c p k n -> p c k n"), ysb[:]
                        )
                if ch == CHG - 1:
                    del st[it]

            for pi in range(NIT // 2):
                e, o = 2 * pi, 2 * pi + 1
                e_dma(e)
                e_dma(o)
                e_f1(e)
                e_f1(o)
                e_t1(e)
                e_f2(e)
                e_t1(o)
                e_f2(o)
                e_pw(e)
                e_i1(e)
                e_pw(o)
                e_i1(o)
                if pi < NIT // 2 - 1:
                    e_t2(e)
                    e_i2(e)
                    e_t2(o)
                    e_i2(o)
                else:
                    e_t2_half(e, 0)
                    e_i2(e, range(0, CHG // 2))
                    e_t2_half(e, 1)
                    e_i2(e, range(CHG // 2, CHG))
                    e_t2_half(o, 0)
                    e_i2(o, range(0, CHG // 2))
                    e_t2_half(o, 1)
                    e_i2(o, range(CHG // 2, CHG))

    nc.compile()
    return nc


def _host_arrays():
    cst = _consts()
    F_cos, F_sin = cst["F_cos"], cst["F_sin"]
    Tw_cos, Tw_sin = cst["Tw_cos"], cst["Tw_sin"]
    f16 = np.float16
    cosF, sinF = F_cos[:64, :], F_sin[:64, :]
    arrs = {}
    # stacked F1 moving: rows 0:64 act on z_re, rows 64:128 on z_im
    arrs["f1m"] = np.block([[cosF, -sinF], [sinF, cosF]]).astype(f16)
    arrs["f2c"] = F_cos.astype(f16)
    arrs["f2s"] = F_sin.astype(f16)
    arrs["f2sn"] = (-F_sin).astype(f16)
    arrs["fim"] = np.concatenate(
        [F_cos, F_sin, -F_sin, F_cos], axis=1
    ).astype(f16)
    # stacked I2 stationaries: out partitions 0:64 = Re (y even batch),
    # 64:128 = Im (y odd batch)
    arrs["gcs"] = np.concatenate(
        [F_cos[:, :64], F_sin[:, :64]], axis=1
    ).astype(f16)
    arrs["gnc"] = np.concatenate(
        [-F_sin[:, :64], F_cos[:, :64]], axis=1
    ).astype(f16)
    arrs["gncn"] = np.concatenate(
        [F_sin[:, :64], -F_cos[:, :64]], axis=1
    ).astype(f16)
    arrs["t1re"] = np.stack([Tw_cos, Tw_cos], axis=1).astype(f16)
    arrs["t1im"] = np.stack([-Tw_sin, -Tw_sin], axis=1).astype(f16)
    return arrs


def _prep_inputs(x, filt):
    """Full inputs -> list of per-core input maps."""
    consts = _host_arrays()

    kpad = np.zeros((D, NFFT), np.float64)
    kpad[:, :L] = filt
    Kf = (np.fft.fft(kpad, axis=1) / NFFT).reshape(D, 128, 128)  # [c, k2, k1]

    # x -> (D, 2pk, 2ri, 64 n1, 128 n2)
    xq = np.ascontiguousarray(x.transpose(2, 0, 1)).reshape(D, 2, 2, 64, 128)

    in_maps = []
    for ci in range(NC):
        sl = slice(ci * CPC, (ci + 1) * CPC)
        m = dict(consts)
        xc = xq[sl].reshape(NIT, CHG, 2, 2, 64, 128)
        # -> (it, (ri,n1), (chl,pk), n2)
        m["xw"] = np.ascontiguousarray(
            xc.transpose(0, 3, 4, 1, 2, 5).reshape(NIT, 128, 2 * CHG, 128)
        ).astype(np.float16)
        kc = Kf[sl]
        kri = np.stack([kc.real, kc.imag], axis=1)  # (CPC, 2ri, 128k2, 128k1)
        m["kk"] = np.ascontiguousarray(
            kri.reshape(NIT, CHG, 2, 128, 128).transpose(0, 3, 1, 2, 4)
        ).astype(np.float16)
        in_maps.append(m)
    return in_maps


def _post_outputs(res):
    y = np.empty((B, L, D), np.float32)
    for ci in range(NC):
        sl = slice(ci * CPC, (ci + 1) * CPC)
        # (pair, cl, (sig,n1), pk, n2); c = 2*pair+cl, b = 2*pk+sig,
        # l = 128*n1+n2
        r = res.results[ci]["yw"].astype(np.float32)
        r = r.reshape(CPC // 2, 2, 2, 64, 2, 128)
        r = r.transpose(4, 2, 3, 5, 0, 1).reshape(B, L, CPC)
        y[:, :, sl] = r
    return y


def kernel(x: np.ndarray, filt: np.ndarray) -> np.ndarray:
    from concourse.bass_utils import run_bass_kernel_spmd

    assert x.shape == (B, L, D) and filt.shape == (D, L)
    x = np.ascontiguousarray(x, dtype=np.float32)
    filt = np.ascontiguousarray(filt, dtype=np.float32)

    in_maps = _prep_inputs(x, filt)
    nc = _build_program()
    res = run_bass_kernel_spmd(nc, in_maps, core_ids=list(range(NC)))
    return _post_outputs(res)


def run_profiled(inputs):
    """Build + run with NTFF tracing; returns BassKernelResults (test-only)."""
    from concourse.bass_utils import run_bass_kernel_spmd

    x = np.ascontiguousarray(inputs["x"], dtype=np.float32)
    filt = np.ascontiguousarray(inputs["filt"], dtype=np.float32)
    in_maps = _prep_inputs(x, filt)
    nc = _build_program()
    return run_bass_kernel_spmd(
        nc, in_maps, core_ids=list(range(NC)), trace=True
    )


if __name__ == "__main__":
    rng = np.random.default_rng(0)
    x = rng.standard_normal((B, L, D)).astype(np.float32)
    filt = rng.standard_normal((D, L)).astype(np.float32)
    y = kernel(x, filt)
    print("y", y.shape, y.dtype, float(np.abs(y).max()))
